# revision 45
# baseline (speedup 1.0000x reference)
"""BPS-DenseNet Trainium2 kernel (v2).

kernel(**inputs) -> [32, 512] f32. Shards the BPS distance computation
data-parallel over batch across 8 NeuronCores, AllGathers the BPS features,
then every core computes the (tiny) BN-MLP head redundantly; core 0's output
is returned.

v2 changes vs baseline:
- The 3 hi/lo correction matmul passes (bh*ph + bh*pl + bl*ph) are fused
  into ONE K=15 matmul by duplicating rows: lhsT=[bh;bh;bl], rhs=[ph;pl;ph].
  PE streams each point column once instead of 3x.
- PSUM consumption balanced between ACT (fp32->fp16 evac) and DVE
  (direct PSUM min-reduce), alternating flavor per basis chunk.
- MLP head: bn_stats/bn_aggr for one-op stats, bias+relu fused into one
  DVE tensor_scalar (no bias matmuls), single ACT table set (sqrt_and_others),
  L0/L2/LF x0 k-tiles accumulated under the exposed AllGather.
"""
import os
import sys
import types

sys.path.insert(0, '/opt/trn_rl_repo')
import numpy as np

# --- optional NTFF profile hook (only when BPS_TRACE=1; grading path skips) ---
TRACE = os.environ.get("BPS_TRACE", "0") == "1"
if TRACE:
    import antenv
    _mod = types.ModuleType("antenv.axon_hooks")
    _mod._hook = None
    _mod.set_axon_ntff_profile_hook = lambda h: setattr(_mod, "_hook", h)
    _mod.get_axon_ntff_profile_hook = lambda: _mod._hook
    sys.modules["antenv.axon_hooks"] = _mod
    antenv.axon_hooks = _mod
    from trn_agent_boot.trn_boot import _ntff_profile_via_ctypes
    _mod._hook = _ntff_profile_via_ctypes('/opt/axon/libaxon_pjrt.so')

import concourse.bacc as bacc
import concourse.mybir as mybir
import concourse.tile as tile
from concourse import bass_utils

bass_utils.upload_artifacts = lambda tmpdir: tmpdir

B, N, P, H, E = 32, 2048, 1024, 256, 512
NC = 8
BL = B // NC            # batches per core
MCH = P // 128          # basis chunks
MA = 7                  # chunks in the first (overlapped) AllGather
EPS = 1e-5

F32 = mybir.dt.float32
F16 = mybir.dt.float16
BF16 = mybir.dt.bfloat16

# layer defs: K-tile count and output size
NKT = {"L0": 8, "L1": 2, "L2": 10, "L3": 2, "LF": 12}
HOUT = {"L0": H, "L1": H, "L2": H, "L3": H, "LF": E}
WOFF = {}
_off = 0
for _l in ("L0", "L1", "L2", "L3", "LF"):
    WOFF[_l] = _off
    _off += NKT[_l] * HOUT[_l]
WCOLS = _off

# stat columns in bst/mv/std/rstd/amat/cmat and gpk/bpk
SCOL = {"bn0": 0, "L0": 8, "L1": 10, "L2": 12, "L3": 14, "LF": 16}
NMO = {"L0": 2, "L1": 2, "L2": 2, "L3": 2, "LF": 4}
# bias columns in biaspk
BCOL = {"L0": 0, "L1": 2, "L2": 4, "L3": 6, "LF": 8}

_CACHE = {}


def _build_module(fast_bn):
    """fast_bn: all BN gammas are 1 and betas 0 (true for this model's
    initialization) -> A == rstd, skip the gamma multiply."""
    nc = bacc.Bacc("TRN2", target_bir_lowering=False, debug=False,
                   num_devices=NC)

    # basis15 and pts15 packed in one tensor -> one input DMA
    bp15_d = nc.dram_tensor("bp15", [128, P + N], BF16, kind="ExternalInput")
    wts_d = nc.dram_tensor("wts", [128, WCOLS], F16, kind="ExternalInput")
    biaspk_d = nc.dram_tensor("biaspk", [128, 12], F32, kind="ExternalInput")
    gpk_d = nc.dram_tensor("gpk", [128, 20], F32, kind="ExternalInput")
    bpk_d = nc.dram_tensor("bpk", [128, 20], F32, kind="ExternalInput")
    outT_d = nc.dram_tensor("outT", [E, B], F32, kind="ExternalOutput")

    # partition-major collective layout: each partition's features are one
    # contiguous run, so the gather-back DMA uses 48B descriptors, not 8B
    cc0_in = nc.dram_tensor("cc0_in", [1, 4], F32)
    cc0_out = nc.dram_tensor("cc0_out", [NC, 4], F32, addr_space="Shared")
    cc_inA = nc.dram_tensor("cc_inA", [128, MA * BL], F16)
    cc_outA = nc.dram_tensor("cc_outA", [NC * 128, MA * BL], F16,
                             addr_space="Shared")
    MB = MCH - MA
    cc_inB = nc.dram_tensor("cc_inB", [128, MB * BL], F16)
    cc_outB = nc.dram_tensor("cc_outB", [NC * 128, MB * BL], F16,
                             addr_space="Shared")

    AF = mybir.ActivationFunctionType
    ALU = mybir.AluOpType

    with tile.TileContext(nc) as tc:
        with tc.tile_pool(name="sb", bufs=1) as sb:
            # ---- warm up the collectives subsystem ASAP (hides ~40us).
            # cc0_in content is irrelevant (nobody reads cc0_out) — trigger
            # on garbage immediately, no memset/DMA in front.
            nc.gpsimd.collective_compute(
                "AllGather", ALU.bypass,
                replica_groups=[list(range(NC))],
                ins=[cc0_in.ap().opt()], outs=[cc0_out.ap().opt()])

            # ---- inputs to SBUF (order matters: matmul inputs first) ----
            bp15 = sb.tile([128, P + N], BF16)
            nc.sync.dma_start(bp15[:], bp15_d[:])
            basis15 = bp15[:, 0:P]
            pts15 = bp15[:, P:P + N]
            gpk = sb.tile([128, 20], F32)
            bpk = sb.tile([128, 20], F32)
            biaspk = sb.tile([128, 12], F32)
            nc.sync.dma_start(gpk[:], gpk_d[:])
            nc.sync.dma_start(bpk[:], bpk_d[:])
            nc.sync.dma_start(biaspk[:], biaspk_d[:])
            wts = sb.tile([128, WCOLS], F16)
            nc.sync.dma_start(wts[:], wts_d[:])

            featA = sb.tile([128, MCH, BL], F32)
            featH = sb.tile([128, MCH, BL], F16)
            epsc = sb.tile([128, 1], F32)
            nc.gpsimd.memset(epsc[:], EPS)

            MB = MCH - MA
            featAg = sb.tile([128, NC, MA, BL], F16)
            featBg = sb.tile([128, NC, MB, BL], F16)
            ccA_r = cc_outA.ap().rearrange("(r p) c -> p r c", p=128)
            ccB_r = cc_outB.ap().rearrange("(r p) c -> p r c", p=128)
            featAg_r = [featAg.rearrange("p r m b -> p r (m b)")]

            # ---- BPS: one K=15 matmul per tile; min-reduce via ACT/DVE ----
            with tc.tile_pool(name="psb", bufs=1, space="PSUM") as psb, \
                 tc.tile_pool(name="stg", bufs=1) as stg:

                def emit_consumer(m, j, dps):
                    u = m * BL + j
                    s2 = (u % 8 == 7)
                    if not s2:
                        s16 = stg.tile([128, 4, 512], F16,
                                       tag=f"s{u % 3}", name=f"s{m}_{j}")
                        nc.scalar.activation(s16[:, :, :], dps[:, :, :],
                                             AF.Copy)
                        t0 = stg.tile([128, 2, 512], F16,
                                      tag=f"ta{u % 3}", name=f"ta{m}_{j}")
                        mmu = stg.tile([128, 512], F16, tag=f"mu{u % 3}",
                                       name=f"mu{m}_{j}")
                        nc.vector.tensor_tensor(t0[:, :, :],
                                                s16[:, 0:2, :],
                                                s16[:, 2:4, :], ALU.min)
                        nc.vector.tensor_tensor(mmu[:, :], t0[:, 0, :],
                                                t0[:, 1, :], ALU.min)
                        nc.vector.tensor_reduce(
                            featA[:, m, j:j + 1], mmu[:, :],
                            axis=mybir.AxisListType.X, op=ALU.min)
                    else:
                        s16 = stg.tile([128, 2, 512], F16,
                                       tag=f"u{u % 3}", name=f"u{m}_{j}")
                        r1 = stg.tile([128, 2], F32, tag=f"r{u % 3}",
                                      name=f"r{m}_{j}")
                        nc.scalar.activation(s16[:, :, :],
                                             dps[:, 0:2, :], AF.Copy)
                        nc.vector.tensor_reduce(
                            r1[:, 0:1], dps[:, 2:4, :],
                            axis=mybir.AxisListType.XY, op=ALU.min)
                        mmu = stg.tile([128, 512], F16, tag=f"mu{u % 3}",
                                       name=f"mu{m}_{j}")
                        nc.vector.tensor_tensor(mmu[:, :], s16[:, 0, :],
                                                s16[:, 1, :], ALU.min)
                        nc.vector.tensor_reduce(
                            r1[:, 1:2], mmu[:, :],
                            axis=mybir.AxisListType.X, op=ALU.min)
                        nc.vector.tensor_reduce(
                            featA[:, m, j:j + 1], r1[:, :],
                            axis=mybir.AxisListType.X, op=ALU.min)

                for m in range(MCH):
                    # fill two j-units with interleaved matmuls so each
                    # LDWEIGHTS (row group 32j) pulls ahead under the other
                    # unit's running matmul instead of serializing
                    dtiles = {}
                    for jp in (0, 2):
                        for j in (jp, jp + 1):
                            u = m * BL + j
                            dtiles[j] = psb.tile([128, 4, 512], F32,
                                                 tag=f"d{u % 2}",
                                                 name=f"d{m}_{j}")
                        for t in range(4):
                            for j in (jp, jp + 1):
                                nc.tensor.matmul(
                                    dtiles[j][:, t, :],
                                    basis15[32 * j:32 * j + 15,
                                            m * 128:(m + 1) * 128],
                                    pts15[32 * j:32 * j + 15,
                                          t * 512:(t + 1) * 512],
                                    start=True, stop=True,
                                    tile_position=(32 * j, 0))
                        for j in (jp, jp + 1):
                            emit_consumer(m, j, dtiles[j])
                    if m == MA - 1:
                        # finalize + AllGather chunks [0, MA)
                        nc.vector.tensor_scalar_max(featA[:, 0:MA, :],
                                                    featA[:, 0:MA, :], 0.0)
                        nc.scalar.activation(featH[:, 0:MA, :],
                                             featA[:, 0:MA, :], AF.Sqrt)
                        nc.sync.dma_start(
                            cc_inA.ap().rearrange("p (m b) -> p m b", m=MA),
                            featH[:, 0:MA, :])
                        nc.gpsimd.collective_compute(
                            "AllGather", ALU.bypass,
                            replica_groups=[list(range(NC))],
                            ins=[cc_inA.ap().opt()],
                            outs=[cc_outA.ap().opt()])
                        # emit gather-A DMA now so it isn't queued behind
                        # the (data-stalled) cc_inB DMA on the sync queue
                        nc.sync.dma_start(
                            featAg_r[0], ccA_r[:, :, :])
                # finalize + AllGather chunks [MA, MCH)
                nc.vector.tensor_scalar_max(featA[:, MA:MCH, :],
                                            featA[:, MA:MCH, :], 0.0)
                nc.scalar.activation(featH[:, MA:MCH, :],
                                     featA[:, MA:MCH, :], AF.Sqrt)
                nc.sync.dma_start(
                    cc_inB.ap().rearrange("p (m b) -> p m b", m=MB),
                    featH[:, MA:MCH, :])
                nc.gpsimd.collective_compute(
                    "AllGather", ALU.bypass,
                    replica_groups=[list(range(NC))],
                    ins=[cc_inB.ap().opt()], outs=[cc_outB.ap().opt()])

            # ---- gather feat from both AllGathers ----
            # (gather-A DMA was emitted right after the AG-A collective)
            feat = sb.tile([128, MCH, NC, BL], F16)
            nc.vector.tensor_copy(
                feat[:, 0:MA, :, :].rearrange("p m r b -> p r m b"),
                featAg[:, :, :, :])

            # ================= MLP head =================
            bst = sb.tile([128, 20, 6], F32)
            mv = sb.tile([128, 20, 2], F32)     # (mean, var)
            std = sb.tile([128, 20], F32)
            rstd = sb.tile([128, 20], F32)
            amat = sb.tile([128, 20], F32)
            cmat = sb.tile([128, 20], F32)
            tmp = sb.tile([128, 20], F32)

            x0 = sb.tile([128, 8, B], F16)
            h1 = sb.tile([128, 2, B], F16)
            a1 = sb.tile([128, 2, B], F16)
            h2 = sb.tile([128, 2, B], F16)
            a2 = sb.tile([128, 2, B], F16)
            hf = sb.tile([128, 4, B], F32)
            outT = sb.tile([128, 4, B], F32)

            def bn_coeffs(c0, c1):
                # mv[:, c, :] = (mean, var) -> amat = g*rsqrt(var+eps),
                # cmat = beta - mean*amat
                nc.scalar.activation(std[:, c0:c1], mv[:, c0:c1, 1],
                                     AF.Sqrt, bias=epsc[:, :])
                nc.vector.reciprocal(amat[:, c0:c1], std[:, c0:c1])
                if not fast_bn:
                    nc.vector.tensor_tensor(amat[:, c0:c1], gpk[:, c0:c1],
                                            amat[:, c0:c1], ALU.mult)
                nc.vector.tensor_tensor(tmp[:, c0:c1], mv[:, c0:c1, 0],
                                        amat[:, c0:c1], ALU.mult)
                nc.vector.tensor_tensor(cmat[:, c0:c1], bpk[:, c0:c1],
                                        tmp[:, c0:c1], ALU.subtract)

            def bn0_block(m0, m1):
                for c in range(m0, m1):
                    nc.vector.bn_stats(
                        bst[:, c, :],
                        feat[:, c, :, :].rearrange("p r b -> p (r b)"))
                for c in range(m0, m1):
                    nc.vector.bn_aggr(mv[:, c, :], bst[:, c, :])
                bn_coeffs(m0, m1)
                for c in range(m0, m1):
                    src = feat[:, c, :, :].rearrange("p r b -> p (r b)")
                    if c % 2 == 0:
                        nc.vector.tensor_scalar(
                            out=x0[:, c, :], in0=src,
                            scalar1=amat[:, c:c + 1],
                            scalar2=cmat[:, c:c + 1],
                            op0=ALU.mult, op1=ALU.add)
                    else:
                        nc.scalar.activation(
                            x0[:, c, :], src, AF.Identity,
                            scale=amat[:, c:c + 1], bias=cmat[:, c:c + 1])

            bn0_block(0, MA)

            with tc.tile_pool(name="psm", bufs=1, space="PSUM") as psm:
                x0k = [x0[:, k, :] for k in range(8)]
                a1k = [a1[:, i, :] for i in range(2)]
                a2k = [a2[:, i, :] for i in range(2)]
                KT = {"L0": x0k, "L1": [h1[:, i, :] for i in range(2)],
                      "L2": x0k + a1k, "L3": [h2[:, i, :] for i in range(2)],
                      "LF": x0k + a1k + a2k}
                # one PSUM bank per (layer, mo) accumulation group; tags
                # reused across layers that never accumulate concurrently
                ZTAG = {"L0": ("pA", "pB"), "L1": ("pA", "pB"),
                        "L2": ("pC", "pD"), "L3": ("pC", "pD"),
                        "LF": ("pE", "pF", "pG", "pH")}
                zp = {}

                def zalloc(ln):
                    zp[ln] = [psm.tile([128, B], F32, tag=ZTAG[ln][mo],
                                       name=f"z{ln}{mo}")
                              for mo in range(NMO[ln])]

                for ln in ("L0", "L2", "LF"):
                    zalloc(ln)

                def mm_range(ln, k0, k1):
                    nk = NKT[ln]
                    hout = HOUT[ln]
                    base = WOFF[ln]
                    for k in range(k0, k1):
                        for mo in range(NMO[ln]):
                            nc.tensor.matmul(
                                zp[ln][mo][:, :],
                                wts[:, base + k * hout + mo * 128:
                                    base + k * hout + (mo + 1) * 128],
                                KT[ln][k],
                                start=(k == 0), stop=(k == nk - 1))

                # overlap with AllGather-B: accumulate all x0 k-tiles that
                # arrived with AllGather-A
                for ln in ("L0", "L2", "LF"):
                    mm_range(ln, 0, MA)

                # AllGather-B lands: gather + bn0 tail + finish L0
                nc.sync.dma_start(
                    featBg.rearrange("p r m b -> p r (m b)"), ccB_r)
                nc.vector.tensor_copy(
                    feat[:, MA:MCH, :, :].rearrange("p m r b -> p r m b"),
                    featBg[:, :, :, :])
                bn0_block(MA, MCH)
                for ln in ("L0", "L2", "LF"):
                    mm_range(ln, MA, 8)

                def bn_layer(ln, h, dst):
                    """zp[ln] -> (bias+relu, ACT) -> h -> stats (DVE) ->
                    bn coeffs (DVE) -> apply (ACT) -> dst.
                    relu/apply on ScalarE so the two engines pipeline."""
                    nmo = NMO[ln]
                    sc = SCOL[ln]
                    bc = BCOL[ln]
                    for mo in range(nmo):
                        nc.scalar.activation(
                            h[:, mo, :], zp[ln][mo][:, :], AF.Relu,
                            bias=biaspk[:, bc + mo:bc + mo + 1])
                    for mo in range(nmo):
                        nc.vector.bn_stats(bst[:, sc + mo, :], h[:, mo, :])
                    for mo in range(nmo):
                        nc.vector.bn_aggr(mv[:, sc + mo, :],
                                          bst[:, sc + mo, :])
                    bn_coeffs(sc, sc + nmo)
                    for mo in range(nmo):
                        nc.scalar.activation(
                            dst[:, mo, :], h[:, mo, :], AF.Identity,
                            scale=amat[:, sc + mo:sc + mo + 1],
                            bias=cmat[:, sc + mo:sc + mo + 1])

                bn_layer("L0", h1, h1)
                zalloc("L1")
                mm_range("L1", 0, 2)
                bn_layer("L1", a1, a1)
                mm_range("L2", 8, 10)
                bn_layer("L2", h2, h2)
                zalloc("L3")
                mm_range("L3", 0, 2)
                bn_layer("L3", a2, a2)
                mm_range("LF", 8, 12)
                bn_layer("LF", hf, outT)

            outT_r = outT_d.ap().rearrange("(mo p) b -> p mo b", p=128)
            for mo in range(4):
                nc.sync.dma_start(outT_r[:, mo, :], outT[:, mo, :])

    nc.compile()
    return nc


def _prep_inputs(x, basis, bn0_g, bn0_b, W0, b0, g0, beta0, W1, b1, g1, beta1,
                 W2, b2, g2, beta2, W3, b3, g3, beta3, Wf, bf, gf, betaf):
    import ml_dtypes
    f32 = np.float32
    f16 = np.float16
    bf16 = ml_dtypes.bfloat16
    x = np.asarray(x, f32)
    s = (x.astype(np.float64) ** 2).sum(1).astype(f32)        # [B, N]
    basis = np.asarray(basis, f32)

    basis5 = np.zeros((5, P), f32)
    basis5[0:3] = -2.0 * basis.T
    basis5[3] = 1.0
    basis5[4] = (basis ** 2).sum(1)
    b_h = basis5.astype(bf16)
    b_l = (basis5 - b_h.astype(f32)).astype(bf16)
    # K=15 stationary: [bh; bh; bl], replicated at partition 32j per batch j
    basis15 = np.zeros((128, P), bf16)
    for j in range(BL):
        basis15[32 * j:32 * j + 5] = b_h
        basis15[32 * j + 5:32 * j + 10] = b_h
        basis15[32 * j + 10:32 * j + 15] = b_l

    def ktile_cols(WT, hout):
        nk = WT.shape[0] // 128
        return np.concatenate([WT[k * 128:(k + 1) * 128, :]
                               for k in range(nk)], axis=1)

    wts = np.concatenate([
        ktile_cols(np.ascontiguousarray(W0.T), H),
        ktile_cols(np.ascontiguousarray(W1.T), H),
        ktile_cols(np.ascontiguousarray(W2.T), H),
        ktile_cols(np.ascontiguousarray(W3.T), H),
        ktile_cols(np.ascontiguousarray(Wf.T), E),
    ], axis=1).astype(f16)

    def pk(v, n):
        return np.asarray(v, f32).reshape(n, 128).T

    biaspk = np.concatenate([pk(b0, 2), pk(b1, 2), pk(b2, 2), pk(b3, 2),
                             pk(bf, 4)], axis=1).astype(f32)
    gpk = np.concatenate([pk(bn0_g, 8), pk(g0, 2), pk(g1, 2), pk(g2, 2),
                          pk(g3, 2), pk(gf, 4)], axis=1)
    bpk = np.concatenate([pk(bn0_b, 8), pk(beta0, 2), pk(beta1, 2),
                          pk(beta2, 2), pk(beta3, 2), pk(betaf, 4)], axis=1)

    in_maps = []
    for c in range(NC):
        pts15 = np.zeros((128, N), f32)
        for j in range(BL):
            b = c * BL + j
            p5 = np.zeros((5, N), f32)
            p5[0:3] = x[b]
            p5[3] = s[b]
            p5[4] = 1.0
            p5h = p5.astype(bf16).astype(f32)
            p5l = p5 - p5h
            # K=15 moving: [ph; pl; ph]
            pts15[32 * j:32 * j + 5] = p5h
            pts15[32 * j + 5:32 * j + 10] = p5l
            pts15[32 * j + 10:32 * j + 15] = p5h
        bp15 = np.concatenate([basis15, pts15.astype(bf16)], axis=1)
        in_maps.append({"bp15": bp15,
                        "wts": wts, "biaspk": biaspk, "gpk": gpk, "bpk": bpk})
    return in_maps


LAST_EXEC_NS = None
LAST_PROFILE = None


def kernel(**inputs) -> np.ndarray:
    global LAST_EXEC_NS, LAST_PROFILE
    fast_bn = all(
        np.all(np.asarray(inputs[k]) == 1.0)
        for k in ("bn0_g", "g0", "g1", "g2", "g3", "gf")) and all(
        np.all(np.asarray(inputs[k]) == 0.0)
        for k in ("bn0_b", "beta0", "beta1", "beta2", "beta3", "betaf"))
    key = ("nc", fast_bn)
    if key not in _CACHE:
        _CACHE[key] = _build_module(fast_bn)
    nc = _CACHE[key]
    in_maps = _prep_inputs(**inputs)
    res = bass_utils.run_bass_kernel_spmd(
        nc, in_maps, core_ids=list(range(NC)), trace=TRACE)
    LAST_EXEC_NS = res.exec_time_ns
    LAST_PROFILE = res.profile_json
    outT = res.results[0]["outT"]          # [E, B]
    return np.ascontiguousarray(outT.T)    # [B, E]
